# revision 40
# baseline (speedup 1.0000x reference)
"""Trainium2 Bass kernel for InstructedAttentionPositionScores.

Computes the [1, H, Q, K] attention bias of the reference nn.Module.
Sharding: one head per NeuronCore (8 heads, 8 cores, tensor parallel).

Structure of the per-head [Q, K] output (Q = K = 4708, dim_i = 100):
  rows 0..99                       "instruction" rows
    cols 0..99   : inst block (block-diag intra/inter einsum scores)
    cols 100..   : cic[row] broadcast along columns
  rows 100..4707                   "content" rows (N = 24*24*8 = 4608)
    cols 0..99   : cci[col] broadcast along rows (every row identical)
    cols 100..   : content[i, j] = (rs[hi,hj] + cs[wi,wj] + ds[di,dj]) / 3
                   with i = hi*192 + wi*8 + di  (and same for j)

All einsums are tiny (<=10 MFLOP total) and are done on host in float64;
the device kernel does the memory-bound expansion. The device works in
bf16 (output is cast back to f32 on host; tolerance is 2e-2, bf16 error
here is ~4e-3): halves HBM write traffic.

Each SBUF partition holds RPP consecutive output rows so each DMA
descriptor covers RPP*4708*2 contiguous DRAM bytes (bigger descriptors
amortize per-descriptor DMA-engine overhead):
  content[i, j] = cd[i % 192, j % 192] + rs[i // 192, j // 192]
where cd[a, b] = cs[a//8, b//8] + ds[a%8, b%8] is a [192, 192] pattern.
Pattern tiles cdp[i][p, s, c] = cd[(TILE_ROWS*i + RPP*p + s) % 192, c] are
host precomputed for the 3 distinct row-phase offsets; the rs term is added
as a per-partition scalar (tensor_scalar) per 192-column block (the RPP
rows in a partition never straddle a 192-row block boundary).
"""

import os
from contextlib import ExitStack

import numpy as np

# Problem constants (hardcoded per the harness contract).
H = 8
T = 10
EMB = 64
DIM_Q = 4708
DIM_K = 4708
DIM_I = 100
N_CAT = 10
DH, DW, DD = 24, 24, 8
NCONT = DH * DW * DD          # 4608 content rows/cols
PERIOD = DW * DD              # 192: column pattern period
SCALE = float(EMB) ** -0.5    # 1/8
N_CORES = 8
TOPP = 112                    # top-rows tile partitions: 112 = 7*16 spreads
                              # uniformly over the 16 DMA engines (100 does not)

RPP = 4                       # output rows packed per SBUF partition
TILE_ROWS = 128 * RPP         # content rows covered per tile
NT = NCONT // TILE_ROWS       # content tiles
assert NCONT % TILE_ROWS == 0

_PROGRAM_CACHE = {}
LAST_RESULTS = None  # test harness introspection


def _build_program():
    """Build + compile the (shared, SPMD) Bass program once."""
    import concourse.tile as tile
    from concourse import bacc, mybir

    bf = mybir.dt.bfloat16
    fsc = mybir.dt.float32   # per-partition scalar operands must be f32
    nc = bacc.Bacc("TRN2", debug=False)

    # Constants packed by criticality: bfc (cd patterns + ccir, gates the
    # content tiles) first on the sync ring, f32c (svr) on the act ring.
    # The instruction rows [0:100] are fully precomputed on the host and
    # bounced DRAM->SBUF->DRAM (topin -> outt): the load rides the startup
    # window where the DMA engines would otherwise idle, and no compute
    # gates the store. Padded to TOPP=112 partitions (rows 100..111 junk
    # the host drops) so each DMA spreads uniformly over the 16 engines.
    # bfc cols: [0:1152] the 3 cd-pattern tiles, [1152:1352] ccir.
    BFC_W = 3 * RPP * PERIOD + RPP * DIM_I
    F32C_W = NT * DH
    bfc_d = nc.dram_tensor("bfc", [128, BFC_W], bf, kind="ExternalInput")
    f32c_d = nc.dram_tensor("f32c", [128, F32C_W], fsc, kind="ExternalInput")
    topin_d = nc.dram_tensor("topin", [TOPP * DIM_K], bf, kind="ExternalInput")
    out_d = nc.dram_tensor("out", [NCONT, DIM_K], bf, kind="ExternalOutput")
    outt_d = nc.dram_tensor("outt", [TOPP * DIM_K], bf, kind="ExternalOutput")

    with ExitStack() as ctx:
        tc = ctx.enter_context(tile.TileContext(nc))
        const = ctx.enter_context(tc.tile_pool(name="const", bufs=1))

        bfc = const.tile([128, BFC_W], bf, tag="bfc")
        nc.sync.dma_start(bfc[:], bfc_d.ap())
        f32c = const.tile([128, F32C_W], fsc, tag="f32c")
        nc.scalar.dma_start(f32c[:], f32c_d.ap())
        topin = const.tile([TOPP, DIM_K], bf, tag="topin")
        nc.sync.dma_start(topin[:], topin_d.ap())
        nc.scalar.dma_start(outt_d[0 : TOPP * DIM_K], topin[:])

        W3 = RPP * PERIOD
        cdp = [
            bfc[:, i * W3 : (i + 1) * W3].rearrange("p (s c) -> p s c", s=RPP)
            for i in range(3)
        ]
        ccir = bfc[:, 3 * W3 : 3 * W3 + RPP * DIM_I].rearrange(
            "p (s c) -> p s c", s=RPP
        )
        svr = f32c

        # Content rows [100:4708] in NT tiles of TILE_ROWS rows; partition p
        # of tile t holds output rows 100 + TILE_ROWS*t + RPP*p .. +RPP-1.
        # Output stores alternate between the two HWDGE rings. (gpsimd is
        # ~15x slower than vector at these shapes — measured — so compute
        # is split vector/scalar 2:1 only.)
        outp = ctx.enter_context(tc.tile_pool(name="outp", bufs=16 // RPP))

        # For tile 0, store an early column chunk (cci cols + blocks 0..5)
        # as soon as those ops finish so the DMA stream starts ~3us sooner;
        # the rest of tile 0 follows as a second (column-strided) store.
        SPLITC = DIM_I + 6 * PERIOD

        def content_tile(t, dma_eng):
            o = outp.tile([128, RPP, DIM_K], bf, tag="o")
            nc.vector.tensor_copy(o[:, :, :DIM_I], ccir)
            base = cdp[t % 3]
            for hj in range(DH):
                dst = o[:, :, DIM_I + PERIOD * hj : DIM_I + PERIOD * (hj + 1)]
                sv = svr[:, t * DH + hj : t * DH + hj + 1]
                if hj % 3 == 2:
                    nc.scalar.add(dst, base, sv)
                else:
                    nc.vector.tensor_scalar_add(dst, base, sv)
                if t == 0 and hj == 5:
                    nc.sync.dma_start(
                        out_d[0:TILE_ROWS, 0:SPLITC], o[:, :, 0:SPLITC]
                    )
            r0 = TILE_ROWS * t
            if t == 0:
                nc.sync.dma_start(
                    out_d[0:TILE_ROWS, SPLITC:DIM_K], o[:, :, SPLITC:DIM_K]
                )
            else:
                dma_eng.dma_start(out_d[r0 : r0 + TILE_ROWS, :], o[:])

        content_tile(0, nc.sync)
        for t in range(1, NT):
            content_tile(t, nc.sync if t % 2 == 1 else nc.scalar)

    nc.compile()
    return nc


def _precompute(inputs):
    """Tiny per-head einsums in float64 -> compact device inputs."""
    import ml_dtypes

    bf16 = ml_dtypes.bfloat16
    f64 = np.float64
    g = {k: np.asarray(inputs[k], dtype=f64) for k in (
        "enc_intra", "enc_inter", "enc_cic", "enc_cci",
        "enc_h", "enc_w", "enc_d",
        "w_intra", "w_inter", "w_cic", "w_cci", "w_h", "w_w", "w_d",
    )}

    a_intra = np.einsum("hc,nmc->hnm", g["w_intra"], g["enc_intra"])  # [H,T,T]
    a_inter = np.einsum("hc,nmc->hnm", g["w_inter"], g["enc_inter"])
    intra_t = np.tile(a_intra, (1, N_CAT, N_CAT))                     # [H,100,100]
    inter_t = np.tile(a_inter, (1, N_CAT, N_CAT))
    mask = np.kron(np.eye(N_CAT, dtype=bool), np.ones((T, T), dtype=bool))
    inst = np.where(mask[None], intra_t, inter_t) * SCALE             # [H,100,100]

    cic = np.tile(
        np.einsum("hc,tc->ht", g["w_cic"], g["enc_cic"][:, 0, :]), (1, N_CAT)
    ) * SCALE                                                          # [H,100]
    cci = np.tile(
        np.einsum("hc,tc->ht", g["w_cci"], g["enc_cci"][0]), (1, N_CAT)
    ) * SCALE                                                          # [H,100]

    def rel_scores(w, table, n):
        b = np.einsum("hc,lc->hl", w, table)                 # [H, 2*cap-1]
        cap = (table.shape[0] + 1) // 2
        d = np.arange(n)[None, :] - np.arange(n)[:, None]
        idx = np.clip(d + cap - 1, 0, table.shape[0] - 1)
        return b[:, idx] * (SCALE / 3.0)                     # [H, n, n]

    rs = rel_scores(g["w_h"], g["enc_h"], DH)                # [H,24,24]
    cs = rel_scores(g["w_w"], g["enc_w"], DW)                # [H,24,24]
    ds = rel_scores(g["w_d"], g["enc_d"], DD)                # [H,8,8]

    # cd[h,a,b] = cs[h,a//8,b//8] + ds[h,a%8,b%8]  -> [H,192,192]
    cd = cs.repeat(DD, axis=1).repeat(DD, axis=2) + np.tile(ds, (1, DW, DW))

    # cds[h][i, p, s*192+c] = cd[h, (64*i + RPP*p + s) % 192, c]
    offs = (TILE_ROWS * np.arange(3)) % PERIOD               # row-phase offsets
    p_idx = np.arange(128)
    s_idx = np.arange(RPP)
    rows = (offs[:, None, None] + RPP * p_idx[None, :, None]
            + s_idx[None, None, :]) % PERIOD                 # [3,128,RPP]
    cds = cd[:, rows, :].reshape(H, 3, 128, RPP * PERIOD)

    # svr[h][p, t*24+hj] = rs[h, (TILE_ROWS*t + RPP*p)//192, hj]
    hi = (TILE_ROWS * np.arange(NT)[:, None] + RPP * p_idx[None, :]) // PERIOD
    svr = rs[:, hi, :].transpose(0, 2, 1, 3).reshape(H, 128, NT * DH)

    # Packed device inputs (see _build_program for the layouts).
    W3 = RPP * PERIOD
    BFC_W = 3 * W3 + RPP * DIM_I
    bfc = np.zeros((H, 128, BFC_W), dtype=bf16)
    bfc[:, :, : 3 * W3] = cds.transpose(0, 2, 1, 3).reshape(H, 128, 3 * W3)
    bfc[:, :, 3 * W3 :] = np.broadcast_to(
        cci[:, None, None, :], (H, 128, RPP, DIM_I)
    ).reshape(H, 128, RPP * DIM_I)

    f32c = np.ascontiguousarray(svr.astype(np.float32))

    # Full top block (instruction rows), precomputed in f64 and sent as
    # bf16 for the device passthrough.
    topin = np.zeros((H, TOPP, DIM_K), dtype=bf16)
    topin[:, :DIM_I, :DIM_I] = inst
    topin[:, :DIM_I, DIM_I:] = np.broadcast_to(
        cic[:, :, None], (H, DIM_I, DIM_K - DIM_I)
    )

    in_maps = []
    for h in range(H):
        in_maps.append({
            "bfc": np.ascontiguousarray(bfc[h]),
            "f32c": f32c[h],
            "topin": np.ascontiguousarray(topin[h]).reshape(-1),
        })
    return in_maps


def kernel(**inputs):
    global LAST_RESULTS
    from concourse.bass_utils import run_bass_kernel_spmd

    assert int(inputs.get("dim_q", DIM_Q)) == DIM_Q
    assert int(inputs.get("dim_k", DIM_K)) == DIM_K
    assert int(inputs.get("dim_i", DIM_I)) == DIM_I
    assert int(inputs.get("dim_h", DH)) == DH
    assert int(inputs.get("dim_w", DW)) == DW
    assert int(inputs.get("dim_d", DD)) == DD

    if "nc" not in _PROGRAM_CACHE:
        _PROGRAM_CACHE["nc"] = _build_program()
    nc = _PROGRAM_CACHE["nc"]

    in_maps = _precompute(inputs)
    res = run_bass_kernel_spmd(
        nc,
        in_maps,
        core_ids=list(range(N_CORES)),
        tmpdir=os.environ.get("KERNEL_TRACE_DIR") or None,
    )
    LAST_RESULTS = res
    out = np.empty((H, DIM_Q, DIM_K), dtype=np.float32)
    for c in range(N_CORES):
        out[c, :DIM_I] = np.asarray(res.results[c]["outt"], dtype=np.float32).reshape(
            TOPP, DIM_K
        )[:DIM_I]
        out[c, DIM_I:] = np.asarray(res.results[c]["out"], dtype=np.float32)
    return out[None]  # [1, H, Q, K]
